# revision 1
# baseline (speedup 1.0000x reference)
"""GCN layer (message passing + linear + ReLU) on 8 Trainium2 NeuronCores.

out = relu(((scatter_add(h[src] -> dst) + x) * dis) @ W.T),
h = x * dis,  dis = rsqrt(deg + 1),  deg = in-degree via dst counts.

Strategy (SPMD, one program on 8 cores):
  - Nodes sharded contiguously: core c owns rows [c*6250, (c+1)*6250).
  - Host partitions edges by dst owner and sorts by dst (index-only work);
    degree reaches the device as CSR rowptr slices, so deg = rowptr diff
    and dis = 1/sqrt(deg+1) are computed on device in f32.
  - Each core computes the full h table (f32, 512B rows) into its DRAM,
    then bulk-gathers h[src] for its own edges with dma_gather (SWDGE).
    int16 gather indices can't span 50k rows, so edges are processed in
    two passes against table halves h[:32768] / h[32768:].
  - Scatter-add is done on-chip: edges sorted by dst fall into windows of
    128 owned nodes; per 128-edge chunk a one-hot S[e, slot] matrix is
    built on DVE (slot = dst - window_base, 255 pads mask out) and the
    tensor engine accumulates S.T @ H_chunk into the window's PSUM tile.
  - Finalize per window: (+x, *dis), PE transpose, 128x128 linear with
    W.T, ReLU, store.
Chunk counts per (pass, window) are maxed over cores so the single SPMD
program fits every core; shorter cores pad with slot=255 chunks.
"""
import numpy as np

from concourse import bacc, bass, mybir, tile
from concourse.bass_utils import run_bass_kernel_spmd

F32 = mybir.dt.float32
I32 = mybir.dt.int32
I16 = mybir.dt.int16
AF = mybir.ActivationFunctionType
OP = mybir.AluOpType

N = 50000
E = 600000
D = 128
C = 8                      # cores
NPC = N // C               # 6250 nodes per core
WPC = (NPC + 127) // 128   # 49 windows per core
NPAD = WPC * 128           # 6272 padded shard rows
PT_G = (N + 127) // 128    # 391 global node tiles
NPAD_G = PT_G * 128        # 50048
SPLIT = 32768              # src table split for int16 gather indices
PASS_BOUNDS = [(0, SPLIT), (SPLIT, N)]
GB = 8                     # chunks per dma_gather batch (1024 idxs; >1024
                           # descriptors per SWDGE call crashes the device)
TB = 8                     # node tiles per h-pass DMA


# ---------------------------------------------------------------- host prep
def host_prep(edge_index):
    src = np.asarray(edge_index[0], dtype=np.int64)
    dst = np.asarray(edge_index[1], dtype=np.int64)
    order = np.argsort(dst, kind="stable")
    ss_all = src[order]
    dd_all = dst[order]
    counts = np.bincount(dst, minlength=N)
    rowptr = np.zeros(N + 1, np.int64)
    rowptr[1:] = np.cumsum(counts)

    rp = np.full(NPAD_G + 1, rowptr[N], np.int64)
    rp[: N + 1] = rowptr
    rp0g = rp[:NPAD_G].reshape(PT_G, 128).T.astype(np.int32).copy()
    rp1g = rp[1 : NPAD_G + 1].reshape(PT_G, 128).T.astype(np.int32).copy()

    per_core = []
    need = np.zeros((C, 2, WPC), np.int64)
    for c in range(C):
        e0, e1 = rowptr[c * NPC], rowptr[(c + 1) * NPC]
        ss, dd = ss_all[e0:e1], dd_all[e0:e1]
        per_core.append((ss, dd))
        for p, (lo, hi) in enumerate(PASS_BOUNDS):
            m = (ss >= lo) & (ss < hi)
            w = (dd[m] - c * NPC) // 128
            need[c, p] = np.bincount(w, minlength=WPC)
    K = np.ceil(need.max(axis=0) / 128).astype(np.int64)  # [2, WPC]
    CH = K.sum(axis=1)
    cstart = np.zeros((2, WPC), np.int64)
    for p in range(2):
        cstart[p, 1:] = np.cumsum(K[p][:-1])

    cores = []
    for c in range(C):
        ss, dd = per_core[c]
        d = {}
        for p, (lo, hi) in enumerate(PASS_BOUNDS):
            m = (ss >= lo) & (ss < hi)
            sp = (ss[m] - lo).astype(np.int64)
            dloc = dd[m] - c * NPC
            w = dloc // 128
            g = np.zeros(CH[p] * 128, np.int64)
            s = np.full(CH[p] * 128, 255, np.int64)
            cnt = np.bincount(w, minlength=WPC)
            ofs = np.zeros(WPC, np.int64)
            ofs[1:] = np.cumsum(cnt[:-1])
            pos = cstart[p, w] * 128 + (np.arange(len(sp)) - ofs[w])
            g[pos] = sp
            s[pos] = dloc - w * 128
            tag = "lo" if p == 0 else "hi"
            # gather idx layout [128, CH*8]: stream pos j at [j%16, j//16],
            # replicated across the 8 groups of 16 partitions.
            d[f"gidx_{tag}"] = np.tile(
                g.reshape(-1, 16).T.astype(np.int16), (8, 1)
            ).copy()
            # slot layout [128, CH]: stream pos j at [j%128, j//128]
            d[f"slots_{tag}"] = s.reshape(-1, 128).T.astype(np.int16).copy()
        n0 = c * NPC
        rpv = np.full(NPAD + 1, rowptr[min((c + 1) * NPC, N)], np.int64)
        rpv[: NPC + 1] = rowptr[n0 : n0 + NPC + 1]
        d["rp0s"] = rpv[:NPAD].reshape(WPC, 128).T.astype(np.int32).copy()
        d["rp1s"] = rpv[1 : NPAD + 1].reshape(WPC, 128).T.astype(np.int32).copy()
        cores.append(d)
    return dict(K=K, CH=CH, cores=cores, rp0g=rp0g, rp1g=rp1g)


# ---------------------------------------------------------------- program
def build_program(K):
    import os
    K = np.asarray(K)
    CH = K.sum(axis=1)
    psa_bufs = int(os.environ.get("PSA_BUFS", "2"))
    nc = bacc.Bacc(None, target_bir_lowering=False, debug=False)

    x_p = nc.dram_tensor("x", [NPAD_G, D], F32, kind="ExternalInput")
    xs_p = nc.dram_tensor("xs", [NPAD, D], F32, kind="ExternalInput")
    wt_p = nc.dram_tensor("wt", [D, D], F32, kind="ExternalInput")
    iota_p = nc.dram_tensor("iota", [128, 128], F32, kind="ExternalInput")
    ident_p = nc.dram_tensor("ident", [128, 128], F32, kind="ExternalInput")
    rp0g_p = nc.dram_tensor("rp0g", [128, PT_G], I32, kind="ExternalInput")
    rp1g_p = nc.dram_tensor("rp1g", [128, PT_G], I32, kind="ExternalInput")
    rp0s_p = nc.dram_tensor("rp0s", [128, WPC], I32, kind="ExternalInput")
    rp1s_p = nc.dram_tensor("rp1s", [128, WPC], I32, kind="ExternalInput")
    gidx_p = [
        nc.dram_tensor("gidx_lo", [128, int(CH[0]) * 8], I16, kind="ExternalInput"),
        nc.dram_tensor("gidx_hi", [128, int(CH[1]) * 8], I16, kind="ExternalInput"),
    ]
    slots_p = [
        nc.dram_tensor("slots_lo", [128, int(CH[0])], I16, kind="ExternalInput"),
        nc.dram_tensor("slots_hi", [128, int(CH[1])], I16, kind="ExternalInput"),
    ]
    out_p = nc.dram_tensor("out", [NPAD, D], F32, kind="ExternalOutput")
    h_lo_t = nc.dram_tensor("h_lo", [SPLIT, D], F32)
    h_hi_t = nc.dram_tensor("h_hi", [NPAD_G - SPLIT, D], F32)

    with tile.TileContext(nc) as tc:
        with (
            tc.tile_pool(name="const", bufs=1) as cpool,
            tc.tile_pool(name="hpass", bufs=3) as hpool,
            tc.tile_pool(name="gather", bufs=8) as gpool,
            tc.tile_pool(name="meta", bufs=2) as mpool,
            tc.tile_pool(name="sel", bufs=8) as spool,
            tc.tile_pool(name="fin", bufs=3) as fpool,
            tc.tile_pool(name="psA", bufs=psa_bufs, space="PSUM") as psA,
            tc.tile_pool(name="psT", bufs=2, space="PSUM") as psT,
            tc.tile_pool(name="psO", bufs=2, space="PSUM") as psO,
        ):
            # --- constants
            wt_sb = cpool.tile([128, 128], F32, tag="wt")
            nc.sync.dma_start(wt_sb[:], wt_p[:])
            iota_sb = cpool.tile([128, 128], F32, tag="iota")
            nc.sync.dma_start(iota_sb[:], iota_p[:])
            ident_sb = cpool.tile([128, 128], F32, tag="ident")
            nc.sync.dma_start(ident_sb[:], ident_p[:])

            # --- prefetch gather indices + slot ids (ahead of h-pass in the
            # sync DMA FIFO so the first gathers aren't queued behind it)
            gidx_sb, stf = [], []
            for p in range(2):
                gi = cpool.tile([128, int(CH[p]) * 8], I16, tag=f"gidx{p}")
                nc.sync.dma_start(gi[:], gidx_p[p][:])
                si = mpool.tile([128, int(CH[p])], I16, tag="si")
                nc.sync.dma_start(si[:], slots_p[p][:])
                sf = cpool.tile([128, int(CH[p])], F32, tag=f"sf{p}")
                nc.vector.tensor_copy(sf[:], si[:])
                gidx_sb.append(gi)
                stf.append(sf)

            xs_v = xs_p[:].rearrange("(u p) d -> p u d", p=128)
            xsw = cpool.tile([128, NPAD], F32, tag="xsw")
            nc.sync.dma_start(
                out=xsw[:].rearrange("p (u e) -> p u e", e=128), in_=xs_v[:, :, :]
            )

            # --- dis = 1/sqrt(deg+1) from rowptr diffs
            def compute_dis(rp0_param, rp1_param, T, tag):
                r0i = cpool.tile([128, T], I32, tag=f"{tag}_r0i")
                nc.sync.dma_start(r0i[:], rp0_param[:])
                r1i = cpool.tile([128, T], I32, tag=f"{tag}_r1i")
                nc.sync.dma_start(r1i[:], rp1_param[:])
                r0f = cpool.tile([128, T], F32, tag=f"{tag}_r0f")
                nc.vector.tensor_copy(r0f[:], r0i[:])
                r1f = cpool.tile([128, T], F32, tag=f"{tag}_r1f")
                nc.vector.tensor_copy(r1f[:], r1i[:])
                dg = cpool.tile([128, T], F32, tag=f"{tag}_dg")
                nc.vector.tensor_tensor(out=dg[:], in0=r1f[:], in1=r0f[:], op=OP.subtract)
                nc.vector.tensor_scalar_add(out=dg[:], in0=dg[:], scalar1=1.0)
                rc = cpool.tile([128, T], F32, tag=f"{tag}_rc")
                nc.vector.reciprocal(rc[:], dg[:])
                ds = cpool.tile([128, T], F32, tag=f"{tag}_dis")
                nc.scalar.activation(ds[:], rc[:], AF.Sqrt)
                return ds

            dis_g = compute_dis(rp0g_p, rp1g_p, PT_G, "g")
            dis_s = compute_dis(rp0s_p, rp1s_p, WPC, "s")

            # --- h = x * dis; hi half first so pass-hi gathers start early
            TSPLIT = SPLIT // 128  # 256, multiple of TB
            x_v = x_p[:].rearrange("(t p) d -> p t d", p=128)
            h_lo_v = h_lo_t[:].rearrange("(t p) d -> p t d", p=128)
            h_hi_v = h_hi_t[:].rearrange("(t p) d -> p t d", p=128)
            t0_order = list(range(TSPLIT, PT_G, TB)) + list(range(0, TSPLIT, TB))
            for t0 in t0_order:
                nb = min(TB, PT_G - t0)
                xt = hpool.tile([128, TB * 128], F32, tag="xt")
                nc.sync.dma_start(
                    out=xt[:, : nb * 128].rearrange("p (b e) -> p b e", e=128),
                    in_=x_v[:, t0 : t0 + nb, :],
                )
                ht = hpool.tile([128, TB * 128], F32, tag="ht")
                for j in range(nb):
                    nc.scalar.activation(
                        ht[:, j * 128 : (j + 1) * 128],
                        xt[:, j * 128 : (j + 1) * 128],
                        AF.Copy,
                        scale=dis_g[:, t0 + j : t0 + j + 1],
                    )
                hv = h_lo_v if t0 < TSPLIT else h_hi_v
                tb = t0 if t0 < TSPLIT else t0 - TSPLIT
                nc.sync.dma_start(
                    out=hv[:, tb : tb + nb, :],
                    in_=ht[:, : nb * 128].rearrange("p (b e) -> p b e", e=128),
                )

            # --- aggregation, pass-major: hi first (its table half is
            # written first). Finalize runs as a separate phase after both
            # passes (concurrent finalize proved unstable on HW).
            SB = GB
            tables = [h_lo_t, h_hi_t]
            agg_sb = cpool.tile([128, NPAD], F32, tag="agg")

            for p in (1, 0):
                table = tables[p]
                nch = int(CH[p])
                pos = 0
                for u in range(WPC):
                    Ku = int(K[p][u])
                    sl = slice(u * 128, (u + 1) * 128)
                    if Ku == 0:
                        if p == 1:
                            nc.vector.memset(agg_sb[:, sl], 0.0)
                        else:
                            nc.vector.tensor_tensor(
                                out=agg_sb[:, sl], in0=agg_sb[:, sl], in1=xsw[:, sl], op=OP.add)
                            nc.scalar.activation(
                                agg_sb[:, sl], agg_sb[:, sl], AF.Copy,
                                scale=dis_s[:, u : u + 1])
                        continue
                    ps = psA.tile([128, 128], F32, tag="pacc")
                    for kin in range(Ku):
                        g = pos
                        pos += 1
                        b, kk = divmod(g, SB)
                        if kk == 0:
                            b0 = b * SB
                            B = min(SB, nch - b0)
                            gt = gpool.tile([128, SB * 128], F32, tag="gt")
                            gv = gt[:, : B * 128].rearrange("p (b e) -> p b e", e=128)
                            nc.gpsimd.dma_gather(
                                gv, table[:], gidx_sb[p][:, b0 * 8 : (b0 + B) * 8],
                                B * 128, B * 128, 128,
                            )
                            Sw = spool.tile([128, SB * 128], F32, tag="S")
                            base = stf[p][:, b0 : b0 + B]
                            in0 = bass.AP(base.tensor, base.offset, list(base.ap) + [[0, 128]])
                            ii = iota_sb[:]
                            in1 = bass.AP(ii.tensor, ii.offset, [ii.ap[0], [0, B], ii.ap[1]])
                            nc.vector.tensor_tensor(
                                out=Sw[:, : B * 128].rearrange("p (b e) -> p b e", e=128),
                                in0=in0, in1=in1, op=OP.is_equal,
                            )
                        nc.tensor.matmul(
                            ps[:],
                            lhsT=Sw[:, kk * 128 : (kk + 1) * 128],
                            rhs=gt[:, kk * 128 : (kk + 1) * 128],
                            start=(kin == 0),
                            stop=(kin == Ku - 1),
                        )
                    if p == 1:
                        nc.vector.tensor_copy(agg_sb[:, sl], ps[:])
                    else:
                        nc.vector.tensor_tensor(out=agg_sb[:, sl], in0=agg_sb[:, sl], in1=ps[:], op=OP.add)
                        # fold (+x, *dis) in here so these SBUF-only ops ride
                        # the in-order DVE/ACT streams instead of queuing
                        # after the whole aggregation phase
                        nc.vector.tensor_tensor(
                            out=agg_sb[:, sl], in0=agg_sb[:, sl], in1=xsw[:, sl], op=OP.add)
                        nc.scalar.activation(
                            agg_sb[:, sl], agg_sb[:, sl], AF.Copy,
                            scale=dis_s[:, u : u + 1])

            # --- finalize phase: transpose + linear + relu + store only
            out_v = out_p[:].rearrange("(u p) d -> p u d", p=128)
            for u in range(WPC):
                sl = slice(u * 128, (u + 1) * 128)
                pt = psT.tile([128, 128], F32, tag="pt")
                nc.tensor.transpose(pt[:], agg_sb[:, sl], ident_sb[:])
                att = fpool.tile([128, 128], F32, tag="fat")
                nc.scalar.copy(att[:], pt[:])
                po = psO.tile([128, 128], F32, tag="po")
                nc.tensor.matmul(po[:], lhsT=att[:], rhs=wt_sb[:], start=True, stop=True)
                ot = fpool.tile([128, 128], F32, tag="fo")
                nc.scalar.activation(ot[:], po[:], AF.Relu)
                nc.sync.dma_start(out_v[:, u, :], ot[:])

    nc.compile()
    return nc


# ---------------------------------------------------------------- runner
_CACHE = {}


def _get_program(K):
    key = K.tobytes()
    if key not in _CACHE:
        _CACHE[key] = build_program(K)
    return _CACHE[key]


def make_in_maps(x, W, prep):
    x = np.asarray(x, np.float32)
    Wt = np.ascontiguousarray(np.asarray(W, np.float32).T)
    xpad = np.zeros((NPAD_G, D), np.float32)
    xpad[:N] = x
    iota = np.tile(np.arange(128, dtype=np.float32)[None, :], (128, 1))
    ident = np.eye(128, dtype=np.float32)
    in_maps = []
    for c in range(C):
        cd = prep["cores"][c]
        xs = np.zeros((NPAD, D), np.float32)
        xs[:NPC] = x[c * NPC : (c + 1) * NPC]
        in_maps.append(
            {
                "x": xpad,
                "xs": xs,
                "wt": Wt,
                "iota": iota,
                "ident": ident,
                "rp0g": prep["rp0g"],
                "rp1g": prep["rp1g"],
                "rp0s": cd["rp0s"],
                "rp1s": cd["rp1s"],
                "gidx_lo": cd["gidx_lo"],
                "gidx_hi": cd["gidx_hi"],
                "slots_lo": cd["slots_lo"],
                "slots_hi": cd["slots_hi"],
            }
        )
    return in_maps


def run_spmd(x, edge_index, W, trace=False, **spmd_kwargs):
    prep = host_prep(edge_index)
    nc = _get_program(prep["K"])
    in_maps = make_in_maps(x, W, prep)
    res = run_bass_kernel_spmd(nc, in_maps, list(range(C)), trace=trace, **spmd_kwargs)
    out = np.concatenate([res.results[c]["out"][:NPC] for c in range(C)], axis=0)
    return out.astype(np.float32), res


def kernel(x, edge_index, N=None, W=None, **_):
    out, _res = run_spmd(np.asarray(x), np.asarray(edge_index), np.asarray(W))
    return out



# revision 3
# speedup vs baseline: 2.9978x; 2.9978x over previous
"""GCN layer (message passing + linear + ReLU) on 8 Trainium2 NeuronCores.

out = relu(((scatter_add(h[src] -> dst) + x) * dis) @ W.T),
h = x * dis,  dis = rsqrt(deg + 1),  deg = in-degree via dst counts.

Strategy (SPMD, one program on 8 cores):
  - Nodes sharded contiguously: core c owns rows [c*6250, (c+1)*6250).
  - Host partitions edges by dst owner and sorts by dst (index-only work);
    degree reaches the device as CSR rowptr slices, so deg = rowptr diff
    and dis = 1/sqrt(deg+1) are computed on device in f32.
  - No h table: each edge's h[src] = x[src]*dis[src] is obtained by
    gathering x[src] (bf16, 256B rows) straight from a replicated bf16
    copy of x, with dis[src] folded into the one-hot scatter matrix S
    (S values are host-precomputed per-edge scales -- graph metadata).
  - Scatter-add on-chip: edges sorted by dst fall into windows of 128
    owned nodes; per 128-edge chunk S[e, slot] = dis[src]*(slot==dst-base)
    is built on DVE (iota is_equal + scale multiply, bf16), and the PE
    accumulates gt.T @ S = agg.T [feat, slot] into the window's PSUM.
  - int16 gather indices can't span 50k rows, so each window's edges are
    processed against table halves x[:32768] / x[32768:] (pass lo/hi).
  - Gather calls round-robin over 4 SWDGE queues (each queue is served
    by a different pair of Q7 cores, so descriptor generation overlaps).
  - Finalize per window, fused right after its last matmul: att =
    (psum + xT) in bf16, po = att.T @ W.T via PE (no transpose needed:
    agg is feature-major), out = relu(po * dis_dst) via ACT per-partition
    scale, then DMA out.
Chunk counts per (pass, window) are maxed over cores so the single SPMD
program fits every core; shorter cores pad with slot=255 / idx=0 chunks.
"""
import numpy as np
import ml_dtypes

from concourse import bacc, bass, mybir, tile
from concourse.bass_utils import run_bass_kernel_spmd

F32 = mybir.dt.float32
BF16 = mybir.dt.bfloat16
I32 = mybir.dt.int32
I16 = mybir.dt.int16
AF = mybir.ActivationFunctionType
OP = mybir.AluOpType

N = 50000
E = 600000
D = 128
C = 8                      # cores
NPC = N // C               # 6250 nodes per core
WPC = (NPC + 127) // 128   # 49 windows per core
NPAD = WPC * 128           # 6272 padded shard rows
NT_G = (N + 127) // 128    # 391 global node tiles
NROWS = NT_G * 128         # 50048 padded table rows
SPLIT = 32768              # src table split for int16 gather indices
PASS_BOUNDS = [(0, SPLIT), (SPLIT, N)]
GB = 8                     # max chunks per dma_gather call (1024 idxs; >1024
                           # descriptors per SWDGE call crashes the device)
NQ = 4                     # SWDGE queues used round-robin


# ---------------------------------------------------------------- host prep
def host_prep(edge_index):
    src = np.asarray(edge_index[0], dtype=np.int64)
    dst = np.asarray(edge_index[1], dtype=np.int64)
    order = np.argsort(dst, kind="stable")
    ss_all = src[order]
    dd_all = dst[order]
    counts = np.bincount(dst, minlength=N)
    rowptr = np.zeros(N + 1, np.int64)
    rowptr[1:] = np.cumsum(counts)
    dis = 1.0 / np.sqrt(counts.astype(np.float64) + 1.0)  # rsqrt(deg+1)

    per_core = []
    need = np.zeros((C, 2, WPC), np.int64)
    for c in range(C):
        e0, e1 = rowptr[c * NPC], rowptr[(c + 1) * NPC]
        ss, dd = ss_all[e0:e1], dd_all[e0:e1]
        per_core.append((ss, dd))
        for p, (lo, hi) in enumerate(PASS_BOUNDS):
            m = (ss >= lo) & (ss < hi)
            w = (dd[m] - c * NPC) // 128
            need[c, p] = np.bincount(w, minlength=WPC)
    K = np.ceil(need.max(axis=0) / 128).astype(np.int64)  # [2, WPC]

    # global chunk index base per (pass, window), window-major interleaved:
    # for w: pass0 chunks, pass1 chunks.
    cbase = np.zeros((2, WPC), np.int64)
    cb = 0
    for w in range(WPC):
        for p in range(2):
            cbase[p, w] = cb
            cb += K[p, w]
    TC = cb  # total chunks (== total matmuls)

    cores = []
    for c in range(C):
        ss, dd = per_core[c]
        g = np.zeros(TC * 128, np.int64)
        s = np.full(TC * 128, 255, np.int64)
        sc = np.zeros(TC * 128, np.float64)
        for p, (lo, hi) in enumerate(PASS_BOUNDS):
            m = (ss >= lo) & (ss < hi)
            sg = ss[m]                       # global src id
            sp = sg - lo                     # index into table half
            dloc = dd[m] - c * NPC
            w = dloc // 128
            cnt = np.bincount(w, minlength=WPC)
            ofs = np.zeros(WPC, np.int64)
            ofs[1:] = np.cumsum(cnt[:-1])
            pos = cbase[p, w] * 128 + (np.arange(len(sp)) - ofs[w])
            g[pos] = sp
            s[pos] = dloc - w * 128
            sc[pos] = dis[sg]
        d = {}
        # gather idx layout [128, TC*8]: stream pos j at [j%16, j//16],
        # replicated across the 8 groups of 16 partitions.
        d["gidx"] = np.tile(g.reshape(-1, 16).T.astype(np.int16), (8, 1)).copy()
        # slot / scale layout [128, TC]: stream pos j at [j%128, j//128]
        d["slots"] = s.reshape(-1, 128).T.astype(np.int16).copy()
        d["scales"] = sc.reshape(-1, 128).T.astype(ml_dtypes.bfloat16).copy()
        n0 = c * NPC
        rpv = np.full(NPAD + 1, rowptr[min((c + 1) * NPC, N)], np.int64)
        rpv[: NPC + 1] = rowptr[n0 : n0 + NPC + 1]
        d["rp0s"] = rpv[:NPAD].reshape(WPC, 128).T.astype(np.int32).copy()
        d["rp1s"] = rpv[1 : NPAD + 1].reshape(WPC, 128).T.astype(np.int32).copy()
        cores.append(d)
    return dict(K=K, cbase=cbase, TC=TC, cores=cores)


# ---------------------------------------------------------------- program
def build_program(K):
    K = np.asarray(K)
    cbase = np.zeros((2, WPC), np.int64)
    cb = 0
    for w in range(WPC):
        for p in range(2):
            cbase[p, w] = cb
            cb += K[p, w]
    TC = int(cb)

    nc = bacc.Bacc(
        None, target_bir_lowering=False, debug=False, num_swdge_queues=NQ
    )

    x_p = nc.dram_tensor("xb", [NROWS, D], BF16, kind="ExternalInput")
    xst_p = nc.dram_tensor("xst", [D, NPAD], F32, kind="ExternalInput")
    wt_p = nc.dram_tensor("wt", [D, D], BF16, kind="ExternalInput")
    iota_p = nc.dram_tensor("iota", [128, 128], BF16, kind="ExternalInput")
    rp0s_p = nc.dram_tensor("rp0s", [128, WPC], I32, kind="ExternalInput")
    rp1s_p = nc.dram_tensor("rp1s", [128, WPC], I32, kind="ExternalInput")
    gidx_p = nc.dram_tensor("gidx", [128, TC * 8], I16, kind="ExternalInput")
    slots_p = nc.dram_tensor("slots", [128, TC], I16, kind="ExternalInput")
    scales_p = nc.dram_tensor("scales", [128, TC], BF16, kind="ExternalInput")
    out_p = nc.dram_tensor("out", [NPAD, D], F32, kind="ExternalOutput")

    with tile.TileContext(nc) as tc:
        with (
            tc.tile_pool(name="const", bufs=1) as cpool,
            tc.tile_pool(name="gather", bufs=8) as gpool,
            tc.tile_pool(name="sel", bufs=8) as spool,
            tc.tile_pool(name="fin", bufs=3) as fpool,
            tc.tile_pool(name="psA", bufs=3, space="PSUM") as psA,
            tc.tile_pool(name="psO", bufs=2, space="PSUM") as psO,
        ):
            # --- constants / metadata (queued ahead of everything)
            gidx_sb = cpool.tile([128, TC * 8], I16, tag="gidx")
            nc.sync.dma_start(gidx_sb[:], gidx_p[:])
            si = cpool.tile([128, TC], I16, tag="si")
            nc.sync.dma_start(si[:], slots_p[:])
            scf = cpool.tile([128, TC], BF16, tag="scf")
            nc.sync.dma_start(scf[:], scales_p[:])
            wt_sb = cpool.tile([128, 128], BF16, tag="wt")
            nc.sync.dma_start(wt_sb[:], wt_p[:])
            iota_sb = cpool.tile([128, 128], BF16, tag="iota")
            nc.sync.dma_start(iota_sb[:], iota_p[:])
            xst_sb = cpool.tile([128, NPAD], F32, tag="xst")
            nc.sync.dma_start(xst_sb[:], xst_p[:])

            sf = cpool.tile([128, TC], BF16, tag="sf")
            nc.vector.tensor_copy(sf[:], si[:])

            # --- dis_dst = 1/sqrt(deg+1) from rowptr diffs, [128, WPC] f32
            r0i = cpool.tile([128, WPC], I32, tag="r0i")
            nc.sync.dma_start(r0i[:], rp0s_p[:])
            r1i = cpool.tile([128, WPC], I32, tag="r1i")
            nc.sync.dma_start(r1i[:], rp1s_p[:])
            r0f = cpool.tile([128, WPC], F32, tag="r0f")
            nc.vector.tensor_copy(r0f[:], r0i[:])
            r1f = cpool.tile([128, WPC], F32, tag="r1f")
            nc.vector.tensor_copy(r1f[:], r1i[:])
            dg = cpool.tile([128, WPC], F32, tag="dg")
            nc.vector.tensor_tensor(out=dg[:], in0=r1f[:], in1=r0f[:], op=OP.subtract)
            nc.vector.tensor_scalar_add(out=dg[:], in0=dg[:], scalar1=1.0)
            rc = cpool.tile([128, WPC], F32, tag="rc")
            nc.vector.reciprocal(rc[:], dg[:])
            dis_s = cpool.tile([128, WPC], F32, tag="dis")
            nc.scalar.activation(dis_s[:], rc[:], AF.Sqrt)

            tables = [x_p[0:SPLIT, :], x_p[SPLIT:NROWS, :]]
            out_v = out_p[:].rearrange("(u p) d -> p u d", p=128)

            qrr = 0
            for w in range(WPC):
                nmm_w = int(K[0, w] + K[1, w])
                mm_w = 0
                ps = psA.tile([128, 128], F32, tag="pacc")
                for p in range(2):
                    Kw = int(K[p, w])
                    c0 = int(cbase[p, w])
                    done = 0
                    while done < Kw:
                        nch = min(GB, Kw - done)
                        cc = c0 + done
                        gt = gpool.tile([128, GB * 128], BF16, tag="gt")
                        gv = gt[:, : nch * 128].rearrange(
                            "p (b e) -> p b e", e=128
                        )
                        nc.gpsimd.dma_gather(
                            gv,
                            tables[p],
                            gidx_sb[:, cc * 8 : (cc + nch) * 8],
                            nch * 128,
                            nch * 128,
                            D,
                            queue_num=qrr % NQ,
                        )
                        qrr += 1
                        # S[e, slot] = dis[src[e]] * (slot == dst_local[e])
                        Sw = spool.tile([128, GB * 128], BF16, tag="S")
                        Swv = Sw[:, : nch * 128].rearrange(
                            "p (b e) -> p b e", e=128
                        )
                        base = sf[:, cc : cc + nch]
                        in0 = bass.AP(
                            base.tensor, base.offset, list(base.ap) + [[0, 128]]
                        )
                        ii = iota_sb[:]
                        in1 = bass.AP(
                            ii.tensor, ii.offset, [ii.ap[0], [0, nch], ii.ap[1]]
                        )
                        nc.vector.tensor_tensor(
                            out=Swv, in0=in0, in1=in1, op=OP.is_equal
                        )
                        scb = scf[:, cc : cc + nch]
                        in2 = bass.AP(
                            scb.tensor, scb.offset, list(scb.ap) + [[0, 128]]
                        )
                        nc.vector.tensor_tensor(
                            out=Swv, in0=Swv, in1=in2, op=OP.mult
                        )
                        for k in range(nch):
                            sl = slice(k * 128, (k + 1) * 128)
                            nc.tensor.matmul(
                                ps[:],
                                lhsT=gt[:, sl],
                                rhs=Sw[:, sl],
                                start=(mm_w == 0),
                                stop=(mm_w == nmm_w - 1),
                            )
                            mm_w += 1
                        done += nch

                # --- finalize window w: att = (agg.T + x.T) bf16;
                # po = att.T @ W.T; out = relu(po * dis_dst)
                wsl = slice(w * 128, (w + 1) * 128)
                att = fpool.tile([128, 128], BF16, tag="att")
                if nmm_w:
                    nc.vector.tensor_tensor(
                        out=att[:], in0=ps[:], in1=xst_sb[:, wsl], op=OP.add
                    )
                else:
                    nc.vector.tensor_copy(att[:], xst_sb[:, wsl])
                po = psO.tile([128, 128], F32, tag="po")
                nc.tensor.matmul(
                    po[:], lhsT=att[:], rhs=wt_sb[:], start=True, stop=True
                )
                ot = fpool.tile([128, 128], F32, tag="ot")
                nc.scalar.activation(
                    ot[:], po[:], AF.Relu, scale=dis_s[:, w : w + 1]
                )
                nc.sync.dma_start(out_v[:, w, :], ot[:])

    nc.compile()
    return nc


# ---------------------------------------------------------------- runner
_CACHE = {}


def _get_program(K):
    key = K.tobytes()
    if key not in _CACHE:
        _CACHE[key] = build_program(K)
    return _CACHE[key]


def make_in_maps(x, W, prep):
    x = np.asarray(x, np.float32)
    xb = np.zeros((NROWS, D), ml_dtypes.bfloat16)
    xb[:N] = x.astype(ml_dtypes.bfloat16)
    Wt = np.ascontiguousarray(np.asarray(W, np.float32).T).astype(
        ml_dtypes.bfloat16
    )
    iota = np.tile(
        np.arange(128, dtype=np.float32)[None, :], (128, 1)
    ).astype(ml_dtypes.bfloat16)
    in_maps = []
    for c in range(C):
        cd = prep["cores"][c]
        xst = np.zeros((D, NPAD), np.float32)
        xst[:, :NPC] = x[c * NPC : (c + 1) * NPC].T
        in_maps.append(
            {
                "xb": xb,
                "xst": xst,
                "wt": Wt,
                "iota": iota,
                "rp0s": cd["rp0s"],
                "rp1s": cd["rp1s"],
                "gidx": cd["gidx"],
                "slots": cd["slots"],
                "scales": cd["scales"],
            }
        )
    return in_maps


def run_spmd(x, edge_index, W, trace=False, **spmd_kwargs):
    prep = host_prep(edge_index)
    nc = _get_program(prep["K"])
    in_maps = make_in_maps(x, W, prep)
    res = run_bass_kernel_spmd(nc, in_maps, list(range(C)), trace=trace, **spmd_kwargs)
    out = np.concatenate([res.results[c]["out"][:NPC] for c in range(C)], axis=0)
    return out.astype(np.float32), res


def kernel(x, edge_index, N=None, W=None, **_):
    out, _res = run_spmd(np.asarray(x), np.asarray(edge_index), np.asarray(W))
    return out


# revision 4
# speedup vs baseline: 3.1267x; 1.0430x over previous
"""GCN layer (message passing + linear + ReLU) on 8 Trainium2 NeuronCores.

out = relu(((scatter_add(h[src] -> dst) + x) * dis) @ W.T),
h = x * dis,  dis = rsqrt(deg + 1),  deg = in-degree via dst counts.

Strategy (SPMD, one program on 8 cores):
  - Nodes sharded contiguously: core c owns rows [c*6250, (c+1)*6250).
  - Host partitions edges by dst owner and sorts by dst (index-only work);
    degree reaches the device as CSR rowptr slices, so deg = rowptr diff
    and dis = 1/sqrt(deg+1) are computed on device in f32.
  - No h table: each edge's h[src] = x[src]*dis[src] is obtained by
    gathering x[src] (bf16, 256B rows) straight from a replicated bf16
    copy of x, with dis[src] folded into the one-hot scatter matrix S
    (S values are host-precomputed per-edge scales -- graph metadata).
  - Scatter-add on-chip: edges sorted by dst fall into windows of 128
    owned nodes; per 128-edge chunk S[e, slot] = dis[src]*(slot==dst-base)
    is built on DVE and the PE accumulates gt.T @ S = agg.T [feat, slot]
    into the window's PSUM bank.
  - S is built in fixed groups of 8 chunks, stored column-major
    (S[p, col*8 + chunk]) so every DVE operand has a packed last dim and
    the 16-bit 2x DVE mode engages; the matmul rhs reads its chunk with a
    stride-8 AP (free for PE: partition dim is the parallel dim).
  - int16 gather indices can't span 50k rows, so each window's edges are
    processed against table halves x[:32768] / x[32768:] (pass lo/hi).
  - Gather calls round-robin over 4 SWDGE queues (each queue is served
    by a different pair of Q7 cores, so descriptor generation overlaps).
  - gidx/xst uploads are split so the first gather/finalize only waits
    for a small first segment.
  - Finalize per window, fused right after its last matmul: att =
    (psum + xT) in bf16, po = att.T @ W.T via PE (no transpose needed:
    agg is feature-major), out = relu(po * dis_dst) via ACT per-partition
    scale, then DMA out.
Chunk counts per (pass, window) are maxed over cores so the single SPMD
program fits every core; shorter cores pad with slot=255 / idx=0 chunks.
"""
import numpy as np
import ml_dtypes

from concourse import bacc, bass, mybir, tile
from concourse.bass_utils import run_bass_kernel_spmd

F32 = mybir.dt.float32
BF16 = mybir.dt.bfloat16
I32 = mybir.dt.int32
I16 = mybir.dt.int16
AF = mybir.ActivationFunctionType
OP = mybir.AluOpType

N = 50000
E = 600000
D = 128
C = 8                      # cores
NPC = N // C               # 6250 nodes per core
WPC = (NPC + 127) // 128   # 49 windows per core
NPAD = WPC * 128           # 6272 padded shard rows
NT_G = (N + 127) // 128    # 391 global node tiles
NROWS = NT_G * 128         # 50048 padded table rows
SPLIT = 32768              # src table split for int16 gather indices
PASS_BOUNDS = [(0, SPLIT), (SPLIT, N)]
GB = 8                     # max chunks per dma_gather call (1024 idxs; >1024
                           # descriptors per SWDGE call crashes the device)
SG = 8                     # chunks per S-group build
NQ = 4                     # SWDGE queues used round-robin
GIDX_PARTS = 4             # gidx upload split (by window range)
XST_PARTS = 7              # xst upload split (7 windows each)


def _chunk_layout(K):
    """Global chunk index base per (pass, window), window-major interleaved."""
    K = np.asarray(K)
    cbase = np.zeros((2, WPC), np.int64)
    cb = 0
    for w in range(WPC):
        for p in range(2):
            cbase[p, w] = cb
            cb += K[p, w]
    return cbase, int(cb)


# ---------------------------------------------------------------- host prep
def host_prep(edge_index):
    src = np.asarray(edge_index[0], dtype=np.int64)
    dst = np.asarray(edge_index[1], dtype=np.int64)
    order = np.argsort(dst, kind="stable")
    ss_all = src[order]
    dd_all = dst[order]
    counts = np.bincount(dst, minlength=N)
    rowptr = np.zeros(N + 1, np.int64)
    rowptr[1:] = np.cumsum(counts)
    dis = 1.0 / np.sqrt(counts.astype(np.float64) + 1.0)  # rsqrt(deg+1)

    per_core = []
    need = np.zeros((C, 2, WPC), np.int64)
    for c in range(C):
        e0, e1 = rowptr[c * NPC], rowptr[(c + 1) * NPC]
        ss, dd = ss_all[e0:e1], dd_all[e0:e1]
        per_core.append((ss, dd))
        for p, (lo, hi) in enumerate(PASS_BOUNDS):
            m = (ss >= lo) & (ss < hi)
            w = (dd[m] - c * NPC) // 128
            need[c, p] = np.bincount(w, minlength=WPC)
    K = np.ceil(need.max(axis=0) / 128).astype(np.int64)  # [2, WPC]

    cbase, TC = _chunk_layout(K)
    TC8 = ((TC + SG - 1) // SG) * SG

    cores = []
    for c in range(C):
        ss, dd = per_core[c]
        g = np.zeros(TC * 128, np.int64)
        s = np.full(TC8 * 128, 255, np.int64)
        sc = np.zeros(TC8 * 128, np.float64)
        for p, (lo, hi) in enumerate(PASS_BOUNDS):
            m = (ss >= lo) & (ss < hi)
            sg = ss[m]                       # global src id
            sp = sg - lo                     # index into table half
            dloc = dd[m] - c * NPC
            w = dloc // 128
            cnt = np.bincount(w, minlength=WPC)
            ofs = np.zeros(WPC, np.int64)
            ofs[1:] = np.cumsum(cnt[:-1])
            pos = cbase[p, w] * 128 + (np.arange(len(sp)) - ofs[w])
            g[pos] = sp
            s[pos] = dloc - w * 128
            sc[pos] = dis[sg]
        d = {}
        # gather idx layout [128, TC*8]: stream pos j at [j%16, j//16],
        # replicated across the 8 groups of 16 partitions.
        d["gidx"] = np.tile(g.reshape(-1, 16).T.astype(np.int16), (8, 1)).copy()
        # slot / scale layout [128, TC8]: stream pos j at [j%128, j//128]
        d["slots"] = s.reshape(-1, 128).T.astype(np.int16).copy()
        d["scales"] = sc.reshape(-1, 128).T.astype(ml_dtypes.bfloat16).copy()
        n0 = c * NPC
        rpv = np.full(NPAD + 1, rowptr[min((c + 1) * NPC, N)], np.int64)
        rpv[: NPC + 1] = rowptr[n0 : n0 + NPC + 1]
        d["rp0s"] = rpv[:NPAD].reshape(WPC, 128).T.astype(np.int32).copy()
        d["rp1s"] = rpv[1 : NPAD + 1].reshape(WPC, 128).T.astype(np.int32).copy()
        cores.append(d)
    return dict(K=K, cbase=cbase, TC=TC, TC8=TC8, cores=cores)


def _gidx_parts(K, cbase):
    """Split windows into GIDX_PARTS ranges; return per-part window range
    and chunk base so gather calls can address their part tile."""
    parts = []
    per = (WPC + GIDX_PARTS - 1) // GIDX_PARTS
    for i in range(GIDX_PARTS):
        w0, w1 = i * per, min((i + 1) * per, WPC)
        c0 = int(cbase[0, w0]) if w0 < WPC else 0
        c1 = int(cbase[0, w1]) if w1 < WPC else None
        parts.append((w0, w1, c0, c1))
    # chunk count of part i = (next part's base or TC) - base
    return parts


# ---------------------------------------------------------------- program
def build_program(K):
    K = np.asarray(K)
    cbase, TC = _chunk_layout(K)
    TC8 = ((TC + SG - 1) // SG) * SG
    NSG = TC // SG + (1 if TC % SG else 0)  # S groups actually consumed

    nc = bacc.Bacc(
        None, target_bir_lowering=False, debug=False, num_swdge_queues=NQ
    )

    x_p = nc.dram_tensor("xb", [NROWS, D], BF16, kind="ExternalInput")
    xst_p = nc.dram_tensor("xst", [D, NPAD], F32, kind="ExternalInput")
    wt_p = nc.dram_tensor("wt", [D, D], BF16, kind="ExternalInput")
    iota8_p = nc.dram_tensor("iota8", [128, 128 * SG], BF16, kind="ExternalInput")
    rp0s_p = nc.dram_tensor("rp0s", [128, WPC], I32, kind="ExternalInput")
    rp1s_p = nc.dram_tensor("rp1s", [128, WPC], I32, kind="ExternalInput")
    gidx_p = nc.dram_tensor("gidx", [128, TC * 8], I16, kind="ExternalInput")
    slots_p = nc.dram_tensor("slots", [128, TC8], I16, kind="ExternalInput")
    scales_p = nc.dram_tensor("scales", [128, TC8], BF16, kind="ExternalInput")
    out_p = nc.dram_tensor("out", [NPAD, D], F32, kind="ExternalOutput")

    gparts = _gidx_parts(K, cbase)

    with tile.TileContext(nc) as tc:
        with (
            tc.tile_pool(name="const", bufs=1) as cpool,
            tc.tile_pool(name="gather", bufs=8) as gpool,
            tc.tile_pool(name="sel", bufs=6) as spool,
            tc.tile_pool(name="fin", bufs=3) as fpool,
            tc.tile_pool(name="psA", bufs=3, space="PSUM") as psA,
            tc.tile_pool(name="psO", bufs=2, space="PSUM") as psO,
        ):
            # --- uploads, ordered so early windows unblock first
            gidx_sb = {}
            part_of_w = {}
            for i, (w0, w1, c0, c1) in enumerate(gparts):
                nchunks = (c1 if c1 is not None else TC) - c0
                gt_ = cpool.tile([128, nchunks * 8], I16, tag=f"gidx{i}")
                gidx_sb[i] = (gt_, c0)
                for w in range(w0, w1):
                    part_of_w[w] = i
            # part 0 first
            nc.sync.dma_start(
                gidx_sb[0][0][:], gidx_p[:, gidx_sb[0][1] * 8 : gparts[0][3] * 8]
            )
            si = cpool.tile([128, TC8], I16, tag="si")
            nc.sync.dma_start(si[:], slots_p[:])
            scf = cpool.tile([128, TC8], BF16, tag="scf")
            nc.sync.dma_start(scf[:], scales_p[:])
            iota8_sb = cpool.tile([128, 128 * SG], BF16, tag="iota8")
            nc.sync.dma_start(iota8_sb[:], iota8_p[:])
            wt_sb = cpool.tile([128, 128], BF16, tag="wt")
            nc.sync.dma_start(wt_sb[:], wt_p[:])
            r0i = cpool.tile([128, WPC], I32, tag="r0i")
            nc.sync.dma_start(r0i[:], rp0s_p[:])
            r1i = cpool.tile([128, WPC], I32, tag="r1i")
            nc.sync.dma_start(r1i[:], rp1s_p[:])

            sf = cpool.tile([128, TC8], BF16, tag="sf")
            nc.vector.tensor_copy(sf[:], si[:])

            # dis_dst = 1/sqrt(deg+1) from rowptr diffs, [128, WPC] f32
            r0f = cpool.tile([128, WPC], F32, tag="r0f")
            nc.vector.tensor_copy(r0f[:], r0i[:])
            r1f = cpool.tile([128, WPC], F32, tag="r1f")
            nc.vector.tensor_copy(r1f[:], r1i[:])
            dg = cpool.tile([128, WPC], F32, tag="dg")
            nc.vector.tensor_tensor(out=dg[:], in0=r1f[:], in1=r0f[:], op=OP.subtract)
            nc.vector.tensor_scalar_add(out=dg[:], in0=dg[:], scalar1=1.0)
            rc = cpool.tile([128, WPC], F32, tag="rc")
            nc.vector.reciprocal(rc[:], dg[:])
            dis_s = cpool.tile([128, WPC], F32, tag="dis")
            nc.scalar.activation(dis_s[:], rc[:], AF.Sqrt)

            # xst part 0, remaining gidx parts, remaining xst parts
            xst_sb = cpool.tile([128, NPAD], F32, tag="xst")
            xw = (WPC + XST_PARTS - 1) // XST_PARTS  # windows per xst part
            nc.sync.dma_start(
                xst_sb[:, : xw * 128], xst_p[:, : xw * 128]
            )
            for i in range(1, GIDX_PARTS):
                t, c0 = gidx_sb[i]
                c1 = gparts[i][3] if gparts[i][3] is not None else TC
                nc.sync.dma_start(t[:], gidx_p[:, c0 * 8 : c1 * 8])
            for i in range(1, XST_PARTS):
                a, b = i * xw * 128, min((i + 1) * xw * 128, NPAD)
                nc.sync.dma_start(xst_sb[:, a:b], xst_p[:, a:b])

            tables = [x_p[0:SPLIT, :], x_p[SPLIT:NROWS, :]]
            out_v = out_p[:].rearrange("(u p) d -> p u d", p=128)

            # --- S group builder (col-major: S[p, c*SG + k], k = chunk in group)
            sgroups = {}

            def build_sgroup(gb):
                Sw = spool.tile([128, 128 * SG], BF16, tag="S")
                sw = Sw[:]
                o = gb * SG
                in0 = bass.AP(sf.tensor, sf.offset + o, [sf.ap[0], [0, 128], [1, SG]])
                ii = iota8_sb[:]
                in1 = bass.AP(ii.tensor, ii.offset, [ii.ap[0], [SG, 128], [1, SG]])
                outap = bass.AP(sw.tensor, sw.offset, [sw.ap[0], [SG, 128], [1, SG]])
                nc.vector.tensor_tensor(out=outap, in0=in0, in1=in1, op=OP.is_equal)
                in2 = bass.AP(scf.tensor, scf.offset + o, [scf.ap[0], [0, 128], [1, SG]])
                nc.vector.tensor_tensor(out=outap, in0=outap, in1=in2, op=OP.mult)
                sgroups[gb] = Sw
                return Sw

            qrr = 0
            for w in range(WPC):
                nmm_w = int(K[0, w] + K[1, w])
                mm_w = 0
                ps = psA.tile([128, 128], F32, tag="pacc")
                for p in range(2):
                    Kw = int(K[p, w])
                    c0 = int(cbase[p, w])
                    done = 0
                    while done < Kw:
                        nch = min(GB, Kw - done)
                        cc = c0 + done
                        pi = part_of_w[w]
                        ptile, pbase = gidx_sb[pi]
                        lofs = (cc - pbase) * 8
                        gt = gpool.tile([128, GB * 128], BF16, tag="gt")
                        gv = gt[:, : nch * 128].rearrange(
                            "p (b e) -> p b e", e=128
                        )
                        nc.gpsimd.dma_gather(
                            gv,
                            tables[p],
                            ptile[:, lofs : lofs + nch * 8],
                            nch * 128,
                            nch * 128,
                            D,
                            queue_num=qrr % NQ,
                        )
                        qrr += 1
                        for k in range(nch):
                            g = cc + k
                            gb, kk = divmod(g, SG)
                            Sw = sgroups.get(gb)
                            if Sw is None:
                                Sw = build_sgroup(gb)
                            sw = Sw[:]
                            rhs = bass.AP(
                                sw.tensor, sw.offset + kk, [sw.ap[0], [SG, 128]]
                            )
                            nc.tensor.matmul(
                                ps[:],
                                lhsT=gt[:, k * 128 : (k + 1) * 128],
                                rhs=rhs,
                                start=(mm_w == 0),
                                stop=(mm_w == nmm_w - 1),
                            )
                            mm_w += 1
                        done += nch

                # --- finalize window w
                wsl = slice(w * 128, (w + 1) * 128)
                att = fpool.tile([128, 128], BF16, tag="att")
                if nmm_w:
                    nc.vector.tensor_tensor(
                        out=att[:], in0=ps[:], in1=xst_sb[:, wsl], op=OP.add
                    )
                else:
                    nc.vector.tensor_copy(att[:], xst_sb[:, wsl])
                po = psO.tile([128, 128], F32, tag="po")
                nc.tensor.matmul(
                    po[:], lhsT=att[:], rhs=wt_sb[:], start=True, stop=True
                )
                ot = fpool.tile([128, 128], F32, tag="ot")
                nc.scalar.activation(
                    ot[:], po[:], AF.Relu, scale=dis_s[:, w : w + 1]
                )
                nc.sync.dma_start(out_v[:, w, :], ot[:])

    nc.compile()
    return nc


# ---------------------------------------------------------------- runner
_CACHE = {}


def _get_program(K):
    key = K.tobytes()
    if key not in _CACHE:
        _CACHE[key] = build_program(K)
    return _CACHE[key]


def make_in_maps(x, W, prep):
    x = np.asarray(x, np.float32)
    xb = np.zeros((NROWS, D), ml_dtypes.bfloat16)
    xb[:N] = x.astype(ml_dtypes.bfloat16)
    Wt = np.ascontiguousarray(np.asarray(W, np.float32).T).astype(
        ml_dtypes.bfloat16
    )
    # iota8[p, c*SG + k] = c
    iota8 = np.tile(
        np.repeat(np.arange(128, dtype=np.float32), SG)[None, :], (128, 1)
    ).astype(ml_dtypes.bfloat16)
    in_maps = []
    for c in range(C):
        cd = prep["cores"][c]
        xst = np.zeros((D, NPAD), np.float32)
        xst[:, :NPC] = x[c * NPC : (c + 1) * NPC].T
        in_maps.append(
            {
                "xb": xb,
                "xst": xst,
                "wt": Wt,
                "iota8": iota8,
                "rp0s": cd["rp0s"],
                "rp1s": cd["rp1s"],
                "gidx": cd["gidx"],
                "slots": cd["slots"],
                "scales": cd["scales"],
            }
        )
    return in_maps


def run_spmd(x, edge_index, W, trace=False, **spmd_kwargs):
    prep = host_prep(edge_index)
    nc = _get_program(prep["K"])
    in_maps = make_in_maps(x, W, prep)
    res = run_bass_kernel_spmd(nc, in_maps, list(range(C)), trace=trace, **spmd_kwargs)
    out = np.concatenate([res.results[c]["out"][:NPC] for c in range(C)], axis=0)
    return out.astype(np.float32), res


def kernel(x, edge_index, N=None, W=None, **_):
    out, _res = run_spmd(np.asarray(x), np.asarray(edge_index), np.asarray(W))
    return out
